# revision 24
# baseline (speedup 1.0000x reference)
"""Multi-head attention (GAttention) on 8 trn2 NeuronCores.

Reference computation (per batch b):
    q = x @ w_qkv.T            -> [N, 768], heads of 64
    attn = softmax(q k^T / 8)  -> per head [N, M]
    out_h = attn @ v           -> [N, 64]
    out = concat(out_h) @ w_proj.T + b_proj

Sharding: 24 (b, head) units over 8 cores -> each core gets one batch b and
3 heads. Each core computes its heads' attention plus its partial
projection sum [N, 768] (f16); host adds the 4 partials per batch + bias.

Schedule (per core), built around keeping the PE matmul stream dense and
the ScalarE exp stream fed:
  0. 8 junk matmuls at t~3.5us warm the PE HAM clock gate (2.4 GHz) before
     real work; tiny exp warms the ACT table.
  1. qproj (bf16): heads 0|1 stacked in psum partitions -> qT01 [128, N]
     (head0 rows 0:64, head1 rows 64:128) in ONE copy; head 2 duplicated
     via wq2 so qT2 rows 64:128 copy rows 0:64.
  2. attention per quarter (QN=512 queries): first the h01 block (16
     m-tiles, S^T pair = head0 m-tile on PE row group 0 + head1 m-tile on
     row group 64 -> no q duplication), then the h2 block (8 m-tile pairs,
     baseline style). Two S^T pairs issue back-to-back per fused iteration
     to halve the row-split LDWEIGHTS boundary cost.
       S^T -> PSUM [128,2,512]; exp (ACT, fused 0.125 scale) or Schraudolph
       (DVE tensor_scalar -> i16 bitcast bf16) -> SBUF bf16;
       AV: av[128,512] += va^T expT (va = [v | ones]; rows 64:128 of av
       hold the softmax denominator), issued LAG entries behind.
  3. normalize: outTn[0:64] = av[0:64] * recip_approx(av[64:128])
  4. proj (bf16): per 128-row n-tile, heads 0|1 as one 128-deep MM plus
     head 2 on top; staged to f16 and DMA'd per n-tile [128,768].
     proj of quarter q interleaves into quarter q+1's h01 block; qproj of
     quarter q+1 interleaves into quarter q's h2 block.
"""
import numpy as np
import ml_dtypes
from contextlib import ExitStack

import concourse.bass as bass
import concourse.mybir as mybir
import concourse.tile as tile
from concourse import bacc
from concourse.bass_utils import run_bass_kernel_spmd

B, N, DIM = 2, 2048, 768
H, D = 12, 64
M = 2048
NCORES = 8
HPC = 3            # heads per core
NT = N // 128      # 16 query tiles
MT = M // 128      # 16 key tiles
MP = MT // 2       # 8 key-tile pairs
CT = DIM // 128    # 6 contraction tiles for qproj
QN = 512           # attention-unit query granularity (av psum = 1 bank)
NQ = N // QN       # 4 quarters
F32 = mybir.dt.float32
F16 = mybir.dt.float16
BF16 = mybir.dt.bfloat16
I16 = mybir.dt.int16

# Schraudolph fast-exp constants for the DVE offload path, in bf16
# bit-space: i16 = convert(s * A + B); bitcast(i16) as bf16 ~= exp(0.125*s),
# max rel err ~3%. A = 2^7 * 0.125 * log2(e); B = (127 - 0.0436) * 2^7
# (host-tuned minimax shift).
EXP_A = 23.083120654232846
EXP_B = 16250.4192
# which exp tiles run on the DVE instead of ScalarE: per quarter, iteration
# indices (0..11 fused iters: 0-3 = h2 p-pairs, 4-11 = h01 m-pairs), and
# which half (0 = first S^T of the fused pair, 1 = second)
DVE_EXPS = {(1, 0), (3, 0), (5, 1), (7, 1), (9, 1), (11, 1)}

_cached = {}


def build_program():
    nc = bacc.Bacc("TRN2", target_bir_lowering=False, debug=False)
    # DRAM inputs; x is quarter-major so one quarter's x is contiguous, and
    # partition-row-major within a quarter so the DMA byte stream matches
    # the SBUF destination's (partition, c, col) order
    xq_d = nc.dram_tensor("xq", [NQ, 128, CT, QN], BF16, kind="ExternalInput")
    wq01_d = nc.dram_tensor("wq01", [128, CT, 128], BF16,
                            kind="ExternalInput")
    wq2_d = nc.dram_tensor("wq2", [128, CT, 128], BF16,
                           kind="ExternalInput")
    # kT01: rows 0:64 = head0 kT of m-tile m, rows 64:128 = head1 kT
    kT01_d = nc.dram_tensor("kT01", [128, MT, 128], BF16,
                            kind="ExternalInput")
    # kT2: rows 0:64 = head2 kT of even m-tile, 64:128 odd m-tile of pair
    kT2_d = nc.dram_tensor("kT2", [128, MP, 128], BF16, kind="ExternalInput")
    # va01[:, i, m] = [v | ones] of head i, m-tile m (partition = key)
    va01_d = nc.dram_tensor("va01", [128, 2, MT, 128], BF16,
                            kind="ExternalInput")
    va2_d = nc.dram_tensor("va2", [128, MT, 128], BF16, kind="ExternalInput")
    wp01_d = nc.dram_tensor("wp01", [128, DIM], BF16, kind="ExternalInput")
    wp2_d = nc.dram_tensor("wp2", [64, DIM], BF16, kind="ExternalInput")
    out_d = nc.dram_tensor("out", [N, DIM], F16, kind="ExternalOutput")

    with tile.TileContext(nc) as tc, ExitStack() as ctx:
        big = ctx.enter_context(tc.tile_pool(name="big", bufs=1))
        expp = ctx.enter_context(tc.tile_pool(name="expp", bufs=8))
        expi = ctx.enter_context(tc.tile_pool(name="expi", bufs=3))
        nrm = ctx.enter_context(tc.tile_pool(name="nrm", bufs=3))
        stg = ctx.enter_context(tc.tile_pool(name="stg", bufs=3))

        # ACT table warmup: a tiny exp at t~0 so the ~2.7us table load is
        # off the critical path of the first real exp
        wu = big.tile([128, 8], F32)
        nc.gpsimd.memset(wu[:], 0.0)
        wu2 = big.tile([128, 8], F32)
        nc.scalar.activation(wu2[:], wu[:], mybir.ActivationFunctionType.Exp)

        # junk operand for the PE HAM warmup matmuls
        junk = big.tile([128, 512], BF16)
        nc.gpsimd.memset(junk[:], 0.0)

        # persistent SBUF tensors
        wq01_t = big.tile([128, CT, 128], BF16)
        wq2_t = big.tile([128, CT, 128], BF16)
        xT_t = big.tile([128, CT, N], BF16)
        kT01_t = big.tile([128, MT, 128], BF16)
        kT2_t = big.tile([128, MP, 128], BF16)
        va01_t = big.tile([128, 2, MT, 128], BF16)
        va2_t = big.tile([128, MT, 128], BF16)
        wp01_t = big.tile([128, DIM], BF16)
        wp2_t = big.tile([64, DIM], BF16)

        def _dma_xq(q, eng=None):
            (eng or nc.sync).dma_start(xT_t[:, :, q * QN:(q + 1) * QN],
                                       xq_d[q])

        def _dma_xq_pair(q, c0):
            nc.sync.dma_start(xT_t[:, c0:c0 + 2, q * QN:(q + 1) * QN],
                              xq_d[q, :, c0:c0 + 2])

        # DMA order = consumption order, single queue so HBM bandwidth is
        # allocated strictly by need: the h2 attention block runs first
        # (it only needs wq2+kT2+va2), h01 data follows.
        nc.sync.dma_start(wq2_t[:], wq2_d[:])
        _dma_xq_pair(0, 0)
        _dma_xq_pair(0, 2)
        _dma_xq_pair(0, 4)
        nc.sync.dma_start(kT2_t[:], kT2_d[:])
        nc.sync.dma_start(va2_t[:], va2_d[:])
        nc.sync.dma_start(wq01_t[:], wq01_d[:])
        nc.sync.dma_start(kT01_t[:], kT01_d[:])
        nc.sync.dma_start(va01_t[:], va01_d[:])
        _dma_xq(1)
        nc.sync.dma_start(wp01_t[:], wp01_d[:])
        nc.sync.dma_start(wp2_t[:], wp2_d[:])
        _dma_xq(2)
        _dma_xq(3)

        # qT01: head0 at rows 0:64, head1 at 64:128; qT2: head2 duplicated
        qT01_t = big.tile([128, N], BF16)
        qT2_t = big.tile([128, N], BF16)
        # proj contraction operands: heads 0|1 stacked on the partition dim,
        # head 2 separate
        outTn01_t = big.tile([128, N], BF16)
        outTn2_t = big.tile([64, N], BF16)

        # PSUM: st 2x2 banks + av 2x1 + pj 2x1 = 8
        with tc.tile_pool(name="st_ps", bufs=2, space="PSUM") as st_ps, \
             tc.tile_pool(name="av_ps", bufs=2, space="PSUM") as av_ps, \
             tc.tile_pool(name="pj_ps", bufs=2, space="PSUM") as pj_ps:

            # PE HAM warmup: junk matmuls across two psum banks (so they
            # pipeline) start opening the clock gate before the real stream
            jp = st_ps.tile([128, 2, 512], F32, tag="st", name="jp")
            for i in range(8):
                nc.tensor.matmul(jp[:, i % 2], junk[:, 0:128], junk[:],
                                 start=True, stop=True)

            av_by_unit = {}
            pend = []
            proj_todo = []
            proj_staged = {}
            LAG = 4

            def _av(pd):
                # av tiles allocate lazily at the first accumulate so the
                # pool-slot WAR lands after the previous unit's norm reads
                unit, va_ap, et_ap, first, last = pd
                if first:
                    av_by_unit[unit] = av_ps.tile([128, 512], F32,
                                                  tag="av", name="av")
                av = av_by_unit[unit]
                nc.tensor.matmul(av[:], va_ap, et_ap,
                                 start=first, stop=last)
                if last:
                    _norm(unit)
                    h, q = unit
                    if h == 2:
                        proj_staged[q] = [
                            (q, j, oc) for j in range(4) for oc in range(2)]

            def _norm(unit):
                # denominator copied to a base-partition-0 SBUF tile first:
                # the custom-DVE recip misbehaves on HW when its input AP
                # sits at a partition offset (sim-only correct). Last
                # quarter's copies go via ScalarE to shorten the DVE tail.
                h, q = unit
                av = av_by_unit.pop(unit)
                nsl = slice(q * QN, (q + 1) * QN)
                dn = nrm.tile([64, QN], F32, tag="dn", name="dn")
                if q == NQ - 1:
                    nc.scalar.copy(dn[:], av[64:128, :])
                else:
                    nc.vector.tensor_copy(dn[:], av[64:128, :])
                rs = nrm.tile([64, QN], F32, tag="rs", name="rs")
                nc.vector.reciprocal_approx_fast(rs[:], dn[:])
                if h == 0:
                    dst = outTn01_t[0:64, nsl]
                elif h == 1:
                    dst = outTn01_t[64:128, nsl]
                else:
                    dst = outTn2_t[:, nsl]
                nc.vector.tensor_mul(dst, av[0:64, :], rs[:])

            qp_by = {}

            def _qproj_part(q, grp, part):
                # q projection for one (quarter, head-group), 2 c-tiles per
                # call so the PE bubble it injects into the attention stream
                # stays small; accumulates in a pj-pool buffer. grp 0 stacks
                # heads 0|1 in the stationary free dim; grp 1 is head 2
                # duplicated.
                if part == 0:
                    qp_by[(q, grp)] = pj_ps.tile([128, 512], F32,
                                                 tag="pp", name="qp")
                qp = qp_by[(q, grp)]
                wq_t = wq01_t if grp == 0 else wq2_t
                for c in (2 * part, 2 * part + 1):
                    nc.tensor.matmul(
                        qp[:], wq_t[:, c],
                        xT_t[:, c, q * QN:(q + 1) * QN],
                        start=(c == 0), stop=(c == CT - 1),
                    )
                if part == 2:
                    nsl = slice(q * QN, (q + 1) * QN)
                    dst = qT01_t if grp == 0 else qT2_t
                    nc.vector.tensor_copy(dst[:, nsl], qp[:])
                    del qp_by[(q, grp)]

            os_by = {}

            def _proj_half(q, j, oc):
                # one 128-row n-tile x 384 out-cols; heads 0|1 via a single
                # 128-deep contraction, head 2 accumulated on top. Staged to
                # one f16 [128,768] tile per n-tile, DMA'd once.
                nn = (q * 4 + j) * 128
                osl = slice(oc * 384, (oc + 1) * 384)
                pp = pj_ps.tile([128, 512], F32, tag="pp", name="pp")
                nc.tensor.matmul(pp[:, 0:384], outTn01_t[:, nn:nn + 128],
                                 wp01_t[:, osl], start=True, stop=False)
                nc.tensor.matmul(pp[:, 0:384], outTn2_t[:, nn:nn + 128],
                                 wp2_t[:, osl], start=False, stop=True,
                                 tile_position=(0, 0))
                if oc == 0:
                    os_by[(q, j)] = stg.tile([128, DIM], F16, tag="os",
                                             name="os")
                os_t = os_by[(q, j)]
                if oc == 0:
                    nc.vector.tensor_copy(os_t[:, osl], pp[:, 0:384])
                else:
                    nc.scalar.copy(os_t[:, osl], pp[:, 0:384])
                    nc.sync.dma_start(out_d[nn:nn + 128, :], os_t[:])
                    del os_by[(q, j)]

            def _flush(limit):
                while len(pend) > limit:
                    _av(pend.pop(0))

            def _exp(st, on_dve):
                if on_dve:
                    eti = expi.tile([128, 2, 512], I16, tag="eti",
                                    name="eti")
                    nc.vector.tensor_scalar(
                        eti[:], st[:], EXP_A, EXP_B,
                        mybir.AluOpType.mult, mybir.AluOpType.add)
                    return eti[:, 0].bitcast(BF16), eti[:, 1].bitcast(BF16)
                et = expp.tile([128, 2, 512], BF16, tag="et", name="et")
                nc.scalar.activation(
                    et[:], st[:], mybir.ActivationFunctionType.Exp,
                    scale=float(D) ** -0.5,
                )
                return et[:, 0], et[:, 1]

            def _st_h01(q, m):
                # S^T pair: head0 m-tile m on row group 0, head1 on 64
                n0 = q * QN
                st = st_ps.tile([128, 2, 512], F32, tag="st", name="st")
                nc.tensor.matmul(
                    st[:, 0], kT01_t[0:64, m], qT01_t[0:64, n0:n0 + QN],
                    start=True, stop=True, tile_position=(0, 0),
                )
                nc.tensor.matmul(
                    st[:, 1], kT01_t[64:128, m], qT01_t[64:128, n0:n0 + QN],
                    start=True, stop=True, tile_position=(64, 0),
                )
                return st

            def _st_h2(q, p):
                # S^T pair: head2 even m-tile on row group 0, odd on 64
                n0 = q * QN
                st = st_ps.tile([128, 2, 512], F32, tag="st", name="st")
                nc.tensor.matmul(
                    st[:, 0], kT2_t[0:64, p], qT2_t[0:64, n0:n0 + QN],
                    start=True, stop=True, tile_position=(0, 0),
                )
                nc.tensor.matmul(
                    st[:, 1], kT2_t[64:128, p], qT2_t[64:128, n0:n0 + QN],
                    start=True, stop=True, tile_position=(64, 0),
                )
                return st

            # qproj prologue for quarter 0: grp1 (head 2) first — the h2
            # attention block leads each quarter
            for grp in (1, 0):
                for part in range(3):
                    _qproj_part(0, grp, part)

            # fused iterations: per quarter, 4 h2 p-pairs then 8 h01 m-pairs
            # proj of q-1 pops at the top of iters 0..4 (its norms were
            # emitted by q-1's quarter-end flush) plus iters 10..11 where
            # the quarter-end AV drain leaves the PE exp-gated; qproj of
            # q+1 runs 2 parts/iter at iters 8..10
            PROJ_SPREAD = [2, 1, 1, 1, 1, 0, 0, 0, 0, 0, 1, 1]
            QPROJ_PARTS = {8: [(1, 0), (1, 1)], 9: [(1, 2), (0, 0)],
                           10: [(0, 1), (0, 2)]}
            for q in range(NQ):
                proj_todo.extend(proj_staged.pop(q - 1, []))
                for it in range(12):
                    for _ in range(PROJ_SPREAD[it]):
                        if proj_todo:
                            _proj_half(*proj_todo.pop(0))
                    if it < 4:
                        p = 2 * it
                        st_a = _st_h2(q, p)
                        st_b = _st_h2(q, p + 1)
                        _flush(LAG)
                        ea0, ea1 = _exp(st_a, (it, 0) in DVE_EXPS)
                        eb0, eb1 = _exp(st_b, (it, 1) in DVE_EXPS)
                        pend.append(((2, q), va2_t[:, 2 * p], ea0,
                                     p == 0, False))
                        pend.append(((2, q), va2_t[:, 2 * p + 1], ea1,
                                     False, False))
                        pend.append(((2, q), va2_t[:, 2 * p + 2], eb0,
                                     False, False))
                        pend.append(((2, q), va2_t[:, 2 * p + 3], eb1,
                                     False, 2 * p + 3 == MT - 1))
                    else:
                        m = 2 * (it - 4)
                        st_a = _st_h01(q, m)
                        st_b = _st_h01(q, m + 1)
                        _flush(LAG)
                        ea0, ea1 = _exp(st_a, (it, 0) in DVE_EXPS)
                        eb0, eb1 = _exp(st_b, (it, 1) in DVE_EXPS)
                        pend.append(((0, q), va01_t[:, 0, m], ea0,
                                     m == 0, False))
                        pend.append(((1, q), va01_t[:, 1, m], ea1,
                                     m == 0, False))
                        pend.append(((0, q), va01_t[:, 0, m + 1], eb0,
                                     False, m + 1 == MT - 1))
                        pend.append(((1, q), va01_t[:, 1, m + 1], eb1,
                                     False, m + 1 == MT - 1))
                    if q + 1 < NQ:
                        for grp, part in QPROJ_PARTS.get(it, ()):
                            _qproj_part(q + 1, grp, part)
                # quarter-end: drain all AVs so the h01 norms are emitted
                # before the next quarter needs their av-pool slots and
                # proj can pop from iter 0
                _flush(0)
            proj_todo.extend(proj_staged.pop(NQ - 1, []))
            while proj_todo:
                _proj_half(*proj_todo.pop(0))

    nc.compile()
    return nc


def build_in_maps(x, k, v, w_qkv, w_proj):
    x = np.asarray(x, dtype=np.float32)
    k = np.asarray(k, dtype=np.float32)
    v = np.asarray(v, dtype=np.float32)
    wqT = np.ascontiguousarray(np.asarray(w_qkv, np.float32).T)   # [C, 768]
    wpT = np.ascontiguousarray(np.asarray(w_proj, np.float32).T)  # [768, 768]
    bf = ml_dtypes.bfloat16

    in_maps = []
    for core in range(NCORES):
        b = core // 4
        hs = [3 * (core % 4) + i for i in range(HPC)]
        # xq [NQ, 128, CT, QN]: quarter-major, partition-row-major x^T
        xT = x[b].T.astype(bf)                                  # [768, 2048]
        xqm = (xT.reshape(CT, 128, NQ, QN).transpose(2, 1, 0, 3))
        # wq01 [128, CT, 128]: heads 0|1 stacked in the output columns;
        # wq2: head 2 duplicated -> qT rows 64:128 == rows 0:64
        b0 = wqT[:, 64 * hs[0]:64 * hs[0] + 64]
        b1 = wqT[:, 64 * hs[1]:64 * hs[1] + 64]
        b2 = wqT[:, 64 * hs[2]:64 * hs[2] + 64]
        wq01 = (np.concatenate([b0, b1], axis=1)
                .reshape(CT, 128, 128).transpose(1, 0, 2).astype(bf))
        wq2 = (np.concatenate([b2, b2], axis=1)
               .reshape(CT, 128, 128).transpose(1, 0, 2).astype(bf))
        kb = k[b, hs].astype(bf)                                # [3, M, D]
        # kT01 [128, MT, 128]: rows 0:64 head0 kT of m-tile, 64:128 head1
        kT01 = np.empty((128, MT, 128), dtype=bf)
        for m in range(MT):
            kT01[0:64, m, :] = kb[0, 128 * m:128 * m + 128, :].T
            kT01[64:128, m, :] = kb[1, 128 * m:128 * m + 128, :].T
        # kT2 [128, MP, 128]: head2, rows 0:64 even m-tile, 64:128 odd
        kT2 = np.empty((128, MP, 128), dtype=bf)
        for p in range(MP):
            kT2[0:64, p, :] = kb[2, 256 * p:256 * p + 128, :].T
            kT2[64:128, p, :] = kb[2, 256 * p + 128:256 * p + 256, :].T
        # va01 [128, 2, MT, 128], va2 [128, MT, 128]: [v | ones]
        vb = v[b, hs].reshape(HPC, MT, 128, D).transpose(2, 0, 1, 3)
        va01 = np.ones((128, 2, MT, 128), dtype=bf)
        va01[:, 0, :, :D] = vb[:, 0].astype(bf)
        va01[:, 1, :, :D] = vb[:, 1].astype(bf)
        va2 = np.ones((128, MT, 128), dtype=bf)
        va2[:, :, :D] = vb[:, 2].astype(bf)
        # wp01 [128, DIM]: heads 0|1 stacked on partitions; wp2 [64, DIM]
        wp01 = np.empty((128, DIM), dtype=bf)
        wp01[0:64] = wpT[64 * hs[0]:64 * hs[0] + 64, :].astype(bf)
        wp01[64:128] = wpT[64 * hs[1]:64 * hs[1] + 64, :].astype(bf)
        wp2 = np.ascontiguousarray(
            wpT[64 * hs[2]:64 * hs[2] + 64, :].astype(bf))
        in_maps.append({"xq": np.ascontiguousarray(xqm),
                        "wq01": np.ascontiguousarray(wq01),
                        "wq2": np.ascontiguousarray(wq2),
                        "kT01": np.ascontiguousarray(kT01),
                        "kT2": np.ascontiguousarray(kT2),
                        "va01": np.ascontiguousarray(va01),
                        "va2": np.ascontiguousarray(va2),
                        "wp01": wp01, "wp2": wp2})
    return in_maps


def kernel(x, k, v, w_qkv, w_proj, b_proj):
    b_proj = np.asarray(b_proj, dtype=np.float32)

    if "nc" not in _cached:
        _cached["nc"] = build_program()
    nc = _cached["nc"]

    in_maps = build_in_maps(x, k, v, w_qkv, w_proj)
    res = run_bass_kernel_spmd(nc, in_maps, core_ids=list(range(NCORES)))

    out = np.empty((B, N, DIM), dtype=np.float32)
    for b in range(B):
        acc = np.zeros((N, DIM), dtype=np.float32)
        for core in range(4 * b, 4 * b + 4):
            acc += res.results[core]["out"].astype(np.float32)
        out[b] = acc + b_proj
    return out
